# revision 7
# baseline (speedup 1.0000x reference)
"""Cross-attention kernel for 8 Trainium2 NeuronCores (SPMD).

Problem: B=4, T_q=T_kv=2048, Q_DIM=1024, KV_DIM=768, H=16, DK=64, fp32.
  q = q_tokens @ Wq.T ; k = kv_tokens @ Wk.T ; v = kv_tokens @ Wv.T
  out = softmax(q k^T / sqrt(DK)) v @ Wo.T

Sharding (8 cores): core c handles batch b=c//2 and head-group hg=c%2
(8 heads, 512 of the 1024 q-dims).  Each core computes a partial output
projection for its head-group over the full T_q, then a pair-wise
ReduceScatter (cores 2b, 2b+1) sums the partials and leaves each core
with half of the T_q rows for its batch:  core 2b -> t[0:1024],
core 2b+1 -> t[1024:2048].

On-device layout is channel-major ("transposed") end-to-end: all host
inputs are pre-transposed so every matmul contraction dim lands on SBUF
partitions with no device-side transposes:
  xqT [1024,2048], xkvT [768,2048], wqT [1024,512], wkT/wvT [768,512],
  woT [512,1024] (= Wo[:, hg-cols].T)
Softmax runs without max-subtraction (scores are O(6) for randn inputs,
exp is safe in fp32) and the denominator comes free from an appended
ones-column in V during the PV matmul.  All matmuls run as float32r
(full PE rate at moving-dim 512).
"""

import numpy as np

import concourse.bacc as bacc
import concourse.mybir as mybir
import concourse.tile as tile
from concourse import bass_utils

N_CORES = 8
P = 128
TQ = 2048
TKV = 2048
CQ = 1024     # q_tokens channels
CKV = 768     # kv_tokens channels
DQ = 512      # per-core head-group q dims (8 heads x 64)
DK = 64
DO = 1024     # output channels
NJ = 4        # 512-wide t-blocks
NTB = 4       # projection t-blocks
NI = TKV // P  # 16 kv chunks
NHP = DQ // P  # 4 head-pairs
CQ_CH = CQ // P   # 8
CKV_CH = CKV // P  # 6
DO_CH = DO // P   # 8

F32 = mybir.dt.float32
F32R = mybir.dt.float32r
EXP = mybir.ActivationFunctionType.Exp
ADD = mybir.AluOpType.add
MUL = mybir.AluOpType.mult

_compiled = None


def _build():
    nc = bacc.Bacc("TRN2", target_bir_lowering=False, debug=False,
                   num_devices=N_CORES)

    xqT = nc.dram_tensor("xqT", [CQ, TQ], F32R, kind="ExternalInput")
    xkvT = nc.dram_tensor("xkvT", [CKV, TKV], F32R, kind="ExternalInput")
    wqT = nc.dram_tensor("wqT", [CQ, DQ], F32R, kind="ExternalInput")
    wkT = nc.dram_tensor("wkT", [CKV, DQ], F32R, kind="ExternalInput")
    wvT = nc.dram_tensor("wvT", [CKV, DQ], F32R, kind="ExternalInput")
    woT = nc.dram_tensor("woT", [DQ, DO], F32R, kind="ExternalInput")
    onesc = nc.dram_tensor("onesc", [P, 8], F32R, kind="ExternalInput")
    out_ext = nc.dram_tensor("out", [DO, TQ // 2], F32, kind="ExternalOutput")

    groups = [[2 * b, 2 * b + 1] for b in range(N_CORES // 2)]

    with tile.TileContext(nc) as tc:
        with (
            tc.tile_pool(name="weights", bufs=1) as wpool,
            tc.tile_pool(name="xload", bufs=1) as xpool,
            tc.tile_pool(name="stage", bufs=4) as stpool,
            tc.tile_pool(name="attn", bufs=1) as apool,
            tc.tile_pool(name="psum_s", bufs=2, space="PSUM") as ps_s,
            tc.tile_pool(name="psum_pv", bufs=2, space="PSUM") as ps_pv,
            tc.tile_pool(name="psum_u", bufs=2, space="PSUM") as ps_u,
            tc.tile_pool(name="dram", bufs=1, space="DRAM") as dpool,
        ):
            # ---- resident weights ----
            wq_sb = wpool.tile([P, CQ_CH, DQ], F32R, tag="wq")
            wk_sb = wpool.tile([P, CKV_CH, DQ], F32R, tag="wk")
            wv_sb = wpool.tile([P, CKV_CH, DQ], F32R, tag="wv")
            wo_sb = wpool.tile([P, NHP, DO], F32R, tag="wo")
            nc.sync.dma_start(wq_sb[:], wqT.ap().rearrange("(n p) d -> p n d", p=P))
            nc.sync.dma_start(wk_sb[:], wkT.ap().rearrange("(n p) d -> p n d", p=P))
            nc.sync.dma_start(wv_sb[:], wvT.ap().rearrange("(n p) d -> p n d", p=P))
            nc.sync.dma_start(wo_sb[:], woT.ap().rearrange("(n p) d -> p n d", p=P))
            ones_sb = wpool.tile([P, 8, 1], F32R, tag="ones")
            nc.sync.dma_start(ones_sb[:], onesc.ap().rearrange("p (n o) -> p n o", o=1))

            # ---- internal DRAM ----
            qT_d = dpool.tile([NHP, P, TQ], F32R, tag="qT_d")
            kT_d = dpool.tile([NHP, P, TKV], F32R, tag="kT_d")
            v_d = dpool.tile([TKV, 8 * 65], F32R, tag="v_d")
            p_d = dpool.tile([2, DO, TQ // 2], F32, tag="p_d")
            s_d = dpool.tile([DO, TQ // 2], F32, tag="s_d")

            xq_r = xqT.ap().rearrange("(n p) t -> p n t", p=P)
            xkv_r = xkvT.ap().rearrange("(n p) t -> p n t", p=P)
            v_r = v_d[:].rearrange("(n p) d -> p n d", p=P)

            # ================= projections =================
            for tb in range(NTB):
                ts_ = slice(tb * 512, (tb + 1) * 512)
                xkv_t = xpool.tile([P, CKV_CH, 512], F32R, tag="xkv")
                nc.sync.dma_start(xkv_t[:], xkv_r[:, :, ts_])
                xq_t = xpool.tile([P, CQ_CH, 512], F32R, tag="xq")
                nc.sync.dma_start(xq_t[:], xq_r[:, :, ts_])

                # V projection: v[t, dv] for the 4 t-chunks of this block
                for s in range(4):
                    tc_i = tb * 4 + s
                    pv = ps_u.tile([P, 512], F32, tag="u")
                    for c in range(CKV_CH):
                        nc.tensor.matmul(
                            pv[:], xkv_t[:, c, s * P:(s + 1) * P], wv_sb[:, c, :],
                            start=(c == 0), stop=(c == CKV_CH - 1))
                    vst = stpool.tile([P, 8, 65], F32R, tag="vstage")
                    nc.vector.tensor_copy(
                        vst[:, :, 0:64],
                        pv[:].rearrange("p (h d) -> p h d", d=64))
                    nc.vector.tensor_copy(vst[:, :, 64:65], ones_sb[:])
                    nc.sync.dma_start(v_d[tc_i * P:(tc_i + 1) * P, :], vst[:])

                # K/Q projections into DRAM (channel-major, per head-pair)
                for hp in range(NHP):
                    hs = slice(hp * P, (hp + 1) * P)
                    pk = ps_u.tile([P, 512], F32, tag="u")
                    for c in range(CKV_CH):
                        nc.tensor.matmul(
                            pk[:], wk_sb[:, c, hs], xkv_t[:, c, :],
                            start=(c == 0), stop=(c == CKV_CH - 1))
                    kst = stpool.tile([P, 512], F32R, tag="kqstage")
                    nc.vector.tensor_copy(kst[:], pk[:])
                    nc.sync.dma_start(kT_d[hp, :, ts_], kst[:])

                    pq = ps_u.tile([P, 512], F32, tag="u")
                    for c in range(CQ_CH):
                        nc.tensor.matmul(
                            pq[:], wq_sb[:, c, hs], xq_t[:, c, :],
                            start=(c == 0), stop=(c == CQ_CH - 1))
                    qst = stpool.tile([P, 512], F32R, tag="kqstage")
                    nc.vector.tensor_copy(qst[:], pq[:])
                    nc.sync.dma_start(qT_d[hp, :, ts_], qst[:])

            # ================= attention =================
            aoT = apool.tile([P, NHP, TQ], F32R, tag="aoT")
            for hp in range(NHP):
                kt = apool.tile([P, TKV], F32R, tag="kt")
                nc.sync.dma_start(kt[:], kT_d[hp])
                vh = apool.tile([P, NI, 130], F32R, tag="vh")
                nc.sync.dma_start(vh[:], v_r[:, :, hp * 130:(hp + 1) * 130])
                for j in range(NJ):
                    js = slice(j * 512, (j + 1) * 512)
                    qt = apool.tile([P, 512], F32R, tag="qt")
                    nc.sync.dma_start(qt[:], qT_d[hp, :, js])
                    acc_a = ps_pv.tile([P, 512], F32, tag="pv")
                    acc_b = ps_pv.tile([P, 512], F32, tag="pv")
                    for i in range(NI):
                        isl = slice(i * P, (i + 1) * P)
                        sc = ps_s.tile([P, 1024], F32, tag="sc")
                        nc.tensor.matmul(sc[:, 0:512], kt[0:64, isl],
                                         qt[0:64, :], start=True, stop=True)
                        nc.tensor.matmul(sc[:, 512:1024], kt[64:128, isl],
                                         qt[64:128, :], start=True, stop=True)
                        ex = stpool.tile([P, 1024], F32R, tag="ex")
                        nc.scalar.activation(ex[:], sc[:], EXP, scale=0.125)
                        nc.tensor.matmul(acc_a[0:65, :], vh[:, i, 0:65],
                                         ex[:, 0:512],
                                         start=(i == 0), stop=(i == NI - 1))
                        nc.tensor.matmul(acc_b[0:65, :], vh[:, i, 65:130],
                                         ex[:, 512:1024],
                                         start=(i == 0), stop=(i == NI - 1))
                    # normalize: aoT[:, hp, js] = acc[0:64] / acc[64]
                    for half, acc in ((0, acc_a), (1, acc_b)):
                        rec = stpool.tile([P, 512], F32, tag="rec")
                        nc.vector.reciprocal(rec[0:1, :], acc[64:65, :])
                        bc = stpool.tile([P, 512], F32, tag="bc")
                        nc.gpsimd.partition_broadcast(bc[0:64, :], rec[0:1, :],
                                                      channels=64)
                        nc.vector.tensor_tensor(
                            aoT[half * 64:(half + 1) * 64, hp, js],
                            acc[0:64, :], bc[0:64, :], op=MUL)

            # ================= output projection =================
            for do in range(DO_CH):
                ds_ = slice(do * P, (do + 1) * P)
                for j in range(NJ):
                    js = slice(j * 512, (j + 1) * 512)
                    po = ps_u.tile([P, 512], F32, tag="u")
                    for hp in range(NHP):
                        nc.tensor.matmul(po[:], wo_sb[:, hp, ds_],
                                         aoT[:, hp, js],
                                         start=(hp == 0), stop=(hp == NHP - 1))
                    ost = stpool.tile([P, 512], F32, tag="ost")
                    nc.vector.tensor_copy(ost[:], po[:])
                    nc.sync.dma_start(
                        p_d[j // 2, ds_, (j % 2) * 512:(j % 2 + 1) * 512],
                        ost[:])

            # ================= pair ReduceScatter + output =================
            nc.gpsimd.collective_compute(
                "ReduceScatter", ADD, replica_groups=groups,
                ins=[p_d.opt()], outs=[s_d.opt()])
            nc.sync.dma_start(out_ext[:], s_d[:])

    nc.compile()
    return nc


def make_in_maps(q_tokens, kv_tokens, Wq, Wk, Wv, Wo):
    q_tokens = np.asarray(q_tokens, np.float32)
    kv_tokens = np.asarray(kv_tokens, np.float32)
    Wq = np.asarray(Wq, np.float32)
    Wk = np.asarray(Wk, np.float32)
    Wv = np.asarray(Wv, np.float32)
    Wo = np.asarray(Wo, np.float32)
    in_maps = []
    for c in range(N_CORES):
        b, hg = c // 2, c % 2
        sl = slice(hg * DQ, (hg + 1) * DQ)
        in_maps.append({
            "xqT": np.ascontiguousarray(q_tokens[b].T),
            "xkvT": np.ascontiguousarray(kv_tokens[b].T),
            "wqT": np.ascontiguousarray(Wq[sl, :].T),
            "wkT": np.ascontiguousarray(Wk[sl, :].T),
            "wvT": np.ascontiguousarray(Wv[sl, :].T),
            "woT": np.ascontiguousarray(Wo[:, sl].T),
            "onesc": np.ones((P, 8), np.float32),
        })
    return in_maps


def kernel(q_tokens, kv_tokens, Wq, Wk, Wv, Wo):
    global _compiled
    if _compiled is None:
        _compiled = _build()
    nc = _compiled

    in_maps = make_in_maps(q_tokens, kv_tokens, Wq, Wk, Wv, Wo)
    res = bass_utils.run_bass_kernel_spmd(nc, in_maps,
                                          core_ids=list(range(N_CORES)))
    B, half = 4, TQ // 2
    out = np.empty((B, TQ, DO), np.float32)
    for c in range(N_CORES):
        b, h = c // 2, c % 2
        out[b, h * half:(h + 1) * half, :] = res.results[c]["out"].T
    return out


# revision 10
# speedup vs baseline: 1.0576x; 1.0576x over previous
"""Cross-attention kernel for 8 Trainium2 NeuronCores (SPMD).

Problem: B=4, T_q=T_kv=2048, Q_DIM=1024, KV_DIM=768, H=16, DK=64, fp32.
  q = q_tokens @ Wq.T ; k = kv_tokens @ Wk.T ; v = kv_tokens @ Wv.T
  out = softmax(q k^T / sqrt(DK)) v @ Wo.T

Sharding (8 cores): core c handles batch b=c//2 and head-group hg=c%2
(8 heads, 512 of the 1024 q-dims).  Each core computes a partial output
projection for its head-group over the full T_q, then a pair-wise
ReduceScatter (cores 2b, 2b+1) sums the partials and leaves each core
with half of the T_q rows for its batch:  core 2b -> t[0:1024],
core 2b+1 -> t[1024:2048].

On-device layout is channel-major ("transposed") end-to-end: all host
inputs are pre-transposed so every matmul contraction dim lands on SBUF
partitions with no device-side transposes:
  xqT [1024,2048], xkvT [768,2048], wqT [1024,512], wkT/wvT [768,512],
  woT [512,1024] (= Wo[:, hg-cols].T)
Softmax runs without max-subtraction (scores are O(6) for randn inputs,
exp is safe in fp32) and the denominator comes free from an appended
ones-column in V during the PV matmul.  All matmuls run as float32r
(full PE rate at moving-dim 512).
"""

import numpy as np

import concourse.bacc as bacc
import concourse.mybir as mybir
import concourse.tile as tile
from concourse import bass_utils

N_CORES = 8
P = 128
TQ = 2048
TKV = 2048
CQ = 1024     # q_tokens channels
CKV = 768     # kv_tokens channels
DQ = 512      # per-core head-group q dims (8 heads x 64)
DK = 64
DO = 1024     # output channels
NJ = 4        # 512-wide t-blocks
NTB = 4       # projection t-blocks
NI = TKV // P  # 16 kv chunks
NHP = DQ // P  # 4 head-pairs
CQ_CH = CQ // P   # 8
CKV_CH = CKV // P  # 6
DO_CH = DO // P   # 8

F32 = mybir.dt.float32
F32R = mybir.dt.float32r
EXP = mybir.ActivationFunctionType.Exp
ADD = mybir.AluOpType.add
MUL = mybir.AluOpType.mult

_compiled = None


def _build():
    nc = bacc.Bacc("TRN2", target_bir_lowering=False, debug=False,
                   num_devices=N_CORES)

    xqT = nc.dram_tensor("xqT", [CQ, TQ], F32R, kind="ExternalInput")
    xkvT = nc.dram_tensor("xkvT", [CKV, TKV], F32R, kind="ExternalInput")
    wqT = nc.dram_tensor("wqT", [CQ, DQ], F32R, kind="ExternalInput")
    wkT = nc.dram_tensor("wkT", [CKV, DQ], F32R, kind="ExternalInput")
    wvT = nc.dram_tensor("wvT", [CKV, DQ], F32R, kind="ExternalInput")
    woT = nc.dram_tensor("woT", [DQ, DO], F32R, kind="ExternalInput")
    onesc = nc.dram_tensor("onesc", [P, 8], F32R, kind="ExternalInput")
    out_ext = nc.dram_tensor("out", [DO, TQ // 2], F32, kind="ExternalOutput")

    groups = [[2 * b, 2 * b + 1] for b in range(N_CORES // 2)]

    with tile.TileContext(nc) as tc:
        with (
            tc.tile_pool(name="weights", bufs=1) as wpool,
            tc.tile_pool(name="xload", bufs=1) as xpool,
            tc.tile_pool(name="stage", bufs=4) as stpool,
            tc.tile_pool(name="attn", bufs=1) as apool,
            tc.tile_pool(name="dram", bufs=1, space="DRAM") as dpool,
        ):
            # ---- resident weights ----
            wq_sb = wpool.tile([P, CQ_CH, DQ], F32R, tag="wq")
            wk_sb = wpool.tile([P, CKV_CH, DQ], F32R, tag="wk")
            wv_sb = wpool.tile([P, CKV_CH, DQ], F32R, tag="wv")
            wo_sb = wpool.tile([P, NHP, DO], F32R, tag="wo")
            nc.sync.dma_start(wq_sb[:], wqT.ap().rearrange("(n p) d -> p n d", p=P))
            nc.sync.dma_start(wk_sb[:], wkT.ap().rearrange("(n p) d -> p n d", p=P))
            nc.sync.dma_start(wv_sb[:], wvT.ap().rearrange("(n p) d -> p n d", p=P))
            nc.sync.dma_start(wo_sb[:], woT.ap().rearrange("(n p) d -> p n d", p=P))
            ones_sb = wpool.tile([P, 8, 1], F32R, tag="ones")
            nc.sync.dma_start(ones_sb[:], onesc.ap().rearrange("p (n o) -> p n o", o=1))

            # ---- internal DRAM ----
            qT_d = dpool.tile([NHP, P, TQ], F32R, tag="qT_d")
            kT_d = dpool.tile([NHP, P, TKV], F32R, tag="kT_d")
            v_d = dpool.tile([TKV, 8 * 65], F32R, tag="v_d")
            # output partials chunked by pairs of do-tiles so the pair
            # ReduceScatter can start while later do-tiles still compute
            NRS = 4
            RDO = DO // NRS  # 256 do rows per RS chunk
            p_ds = [dpool.tile([2, RDO, TQ // 2], F32, tag=f"p_d{k}",
                               name=f"p_d{k}") for k in range(NRS)]
            s_ds = [dpool.tile([RDO, TQ // 2], F32, tag=f"s_d{k}",
                               name=f"s_d{k}") for k in range(NRS)]

            xq_r = xqT.ap().rearrange("(n p) t -> p n t", p=P)
            xkv_r = xkvT.ap().rearrange("(n p) t -> p n t", p=P)
            v_r = v_d[:].rearrange("(n p) d -> p n d", p=P)

            # ================= projections =================
            with tc.tile_pool(name="psum_proj", bufs=4, space="PSUM") as ps_u:
                for tb in range(NTB):
                    ts_ = slice(tb * 512, (tb + 1) * 512)
                    xkv_t = xpool.tile([P, CKV_CH, 512], F32R, tag="xkv")
                    nc.sync.dma_start(xkv_t[:], xkv_r[:, :, ts_])
                    xq_t = xpool.tile([P, CQ_CH, 512], F32R, tag="xq")
                    nc.sync.dma_start(xq_t[:], xq_r[:, :, ts_])

                    # V projection: v[t, dv] for the 4 t-chunks of this block
                    for s in range(4):
                        tc_i = tb * 4 + s
                        pv = ps_u.tile([P, 512], F32, tag="u")
                        for c in range(CKV_CH):
                            nc.tensor.matmul(
                                pv[:], xkv_t[:, c, s * P:(s + 1) * P],
                                wv_sb[:, c, :],
                                start=(c == 0), stop=(c == CKV_CH - 1))
                        vst = stpool.tile([P, 8, 65], F32R, tag="vstage")
                        nc.vector.tensor_copy(
                            vst[:, :, 0:64],
                            pv[:].rearrange("p (h d) -> p h d", d=64))
                        nc.vector.tensor_copy(vst[:, :, 64:65], ones_sb[:])
                        nc.sync.dma_start(v_d[tc_i * P:(tc_i + 1) * P, :], vst[:])

                    # K/Q projections into DRAM (channel-major, per head-pair)
                    for hp in range(NHP):
                        hs = slice(hp * P, (hp + 1) * P)
                        pk = ps_u.tile([P, 512], F32, tag="u")
                        for c in range(CKV_CH):
                            nc.tensor.matmul(
                                pk[:], wk_sb[:, c, hs], xkv_t[:, c, :],
                                start=(c == 0), stop=(c == CKV_CH - 1))
                        kst = stpool.tile([P, 512], F32R, tag="kqstage")
                        nc.vector.tensor_copy(kst[:], pk[:])
                        nc.sync.dma_start(kT_d[hp, :, ts_], kst[:])

                        pq = ps_u.tile([P, 512], F32, tag="u")
                        for c in range(CQ_CH):
                            nc.tensor.matmul(
                                pq[:], wq_sb[:, c, hs], xq_t[:, c, :],
                                start=(c == 0), stop=(c == CQ_CH - 1))
                        qst = stpool.tile([P, 512], F32R, tag="kqstage")
                        nc.vector.tensor_copy(qst[:], pq[:])
                        nc.sync.dma_start(qT_d[hp, :, ts_], qst[:])

            # ================= attention + output projection =================
            with (
                tc.tile_pool(name="psum_s", bufs=2, space="PSUM") as ps_s,
                tc.tile_pool(name="psum_pv", bufs=4, space="PSUM") as ps_pv,
            ):
                aoT = apool.tile([P, NHP, TQ], F32R, tag="aoT")
                for hp in range(NHP):
                    kt = apool.tile([P, TKV], F32R, tag="kt")
                    nc.sync.dma_start(kt[:], kT_d[hp])
                    vh = apool.tile([P, NI, 130], F32R, tag="vh")
                    nc.sync.dma_start(vh[:], v_r[:, :, hp * 130:(hp + 1) * 130])
                    for j in range(NJ):
                        js = slice(j * 512, (j + 1) * 512)
                        qt = apool.tile([P, 512], F32R, tag="qt")
                        nc.sync.dma_start(qt[:], qT_d[hp, :, js])
                        acc_a = ps_pv.tile([P, 512], F32, tag="pv")
                        acc_b = ps_pv.tile([P, 512], F32, tag="pv")
                        for i in range(NI):
                            isl = slice(i * P, (i + 1) * P)
                            sc = ps_s.tile([P, 1024], F32, tag="sc")
                            nc.tensor.matmul(sc[:, 0:512], kt[0:64, isl],
                                             qt[0:64, :], start=True, stop=True)
                            nc.tensor.matmul(sc[:, 512:1024], kt[64:128, isl],
                                             qt[64:128, :], start=True, stop=True)
                            ex = stpool.tile([P, 1024], F32R, tag="ex")
                            nc.scalar.activation(ex[:], sc[:], EXP, scale=0.125)
                            nc.tensor.matmul(acc_a[0:65, :], vh[:, i, 0:65],
                                             ex[:, 0:512],
                                             start=(i == 0), stop=(i == NI - 1))
                            nc.tensor.matmul(acc_b[0:65, :], vh[:, i, 65:130],
                                             ex[:, 512:1024],
                                             start=(i == 0), stop=(i == NI - 1))
                        # normalize: aoT[:, hp, js] = acc[0:64] / acc[64]
                        for half, acc in ((0, acc_a), (1, acc_b)):
                            rec = stpool.tile([P, 512], F32, tag="rec")
                            nc.vector.reciprocal(rec[0:1, :], acc[64:65, :])
                            bc = stpool.tile([P, 512], F32, tag="bc")
                            nc.gpsimd.partition_broadcast(bc[0:64, :],
                                                          rec[0:1, :],
                                                          channels=64)
                            nc.vector.tensor_tensor(
                                aoT[half * 64:(half + 1) * 64, hp, js],
                                acc[0:64, :], bc[0:64, :], op=MUL)

                # output projection, chunked for early ReduceScatter
                for do in range(DO_CH):
                    ds_ = slice(do * P, (do + 1) * P)
                    k = do // (DO_CH // NRS)
                    rs_ = slice((do % (DO_CH // NRS)) * P,
                                (do % (DO_CH // NRS)) * P + P)
                    for j in range(NJ):
                        js = slice(j * 512, (j + 1) * 512)
                        po = ps_pv.tile([P, 512], F32, tag="pv")
                        for hp in range(NHP):
                            nc.tensor.matmul(po[:], wo_sb[:, hp, ds_],
                                             aoT[:, hp, js],
                                             start=(hp == 0),
                                             stop=(hp == NHP - 1))
                        ost = stpool.tile([P, 512], F32, tag="ost")
                        nc.vector.tensor_copy(ost[:], po[:])
                        nc.sync.dma_start(
                            p_ds[k][j // 2, rs_,
                                    (j % 2) * 512:(j % 2 + 1) * 512],
                            ost[:])
                    if do % (DO_CH // NRS) == (DO_CH // NRS) - 1:
                        nc.gpsimd.collective_compute(
                            "ReduceScatter", ADD, replica_groups=groups,
                            ins=[p_ds[k].opt()], outs=[s_ds[k].opt()])
                        nc.sync.dma_start(
                            out_ext[k * RDO:(k + 1) * RDO, :], s_ds[k][:])

    nc.compile()
    return nc


def make_in_maps(q_tokens, kv_tokens, Wq, Wk, Wv, Wo):
    q_tokens = np.asarray(q_tokens, np.float32)
    kv_tokens = np.asarray(kv_tokens, np.float32)
    Wq = np.asarray(Wq, np.float32)
    Wk = np.asarray(Wk, np.float32)
    Wv = np.asarray(Wv, np.float32)
    Wo = np.asarray(Wo, np.float32)
    in_maps = []
    for c in range(N_CORES):
        b, hg = c // 2, c % 2
        sl = slice(hg * DQ, (hg + 1) * DQ)
        in_maps.append({
            "xqT": np.ascontiguousarray(q_tokens[b].T),
            "xkvT": np.ascontiguousarray(kv_tokens[b].T),
            "wqT": np.ascontiguousarray(Wq[sl, :].T),
            "wkT": np.ascontiguousarray(Wk[sl, :].T),
            "wvT": np.ascontiguousarray(Wv[sl, :].T),
            "woT": np.ascontiguousarray(Wo[:, sl].T),
            "onesc": np.ones((P, 8), np.float32),
        })
    return in_maps


def kernel(q_tokens, kv_tokens, Wq, Wk, Wv, Wo):
    global _compiled
    if _compiled is None:
        _compiled = _build()
    nc = _compiled

    in_maps = make_in_maps(q_tokens, kv_tokens, Wq, Wk, Wv, Wo)
    res = bass_utils.run_bass_kernel_spmd(nc, in_maps,
                                          core_ids=list(range(N_CORES)))
    B, half = 4, TQ // 2
    out = np.empty((B, TQ, DO), np.float32)
    for c in range(N_CORES):
        b, h = c // 2, c % 2
        out[b, h * half:(h + 1) * half, :] = res.results[c]["out"].T
    return out


# revision 13
# speedup vs baseline: 1.4663x; 1.3864x over previous
"""Cross-attention kernel for 8 Trainium2 NeuronCores (SPMD).

Problem: B=4, T_q=T_kv=2048, Q_DIM=1024, KV_DIM=768, H=16, DK=64, fp32.
  q = q_tokens @ Wq.T ; k = kv_tokens @ Wk.T ; v = kv_tokens @ Wv.T
  out = softmax(q k^T / sqrt(DK)) v @ Wo.T

Sharding (8 cores): core c handles batch b=c//2 and head-group hg=c%2
(8 heads, 512 of the 1024 q-dims).  After attention, the pair (2b, 2b+1)
AllGathers the per-head-group attention outputs (one collective per
head-pair chunk, overlapped with the remaining attention work), then each
core runs the output projection against ITS half of the Wo columns —
core c returns out[b, :, (c%2)*512:(c%2+1)*512] transposed.  The
rank-dependent output-channel split lives entirely in the host-side Wo
slice, so the device program is identical on all cores.

On-device layout is channel-major ("transposed") end-to-end: all host
inputs are pre-transposed so every matmul contraction dim lands on SBUF
partitions with no device-side transposes.  Softmax runs without
max-subtraction (scores are O(6) for randn inputs; exp is safe in fp32)
and the denominator comes free from an appended ones-column in V during
the PV matmul.  All matmuls run as float32r (full PE rate at
moving-dim 512); attention score matmuls for the two heads of a pair
run concurrently in the two 64-row halves of the PE array (row tiling).
"""

import numpy as np

import concourse.bacc as bacc
import concourse.mybir as mybir
import concourse.tile as tile
from concourse import bass_utils

N_CORES = 8
P = 128
TQ = 2048
TKV = 2048
CQ = 1024     # q_tokens channels
CKV = 768     # kv_tokens channels
DQ = 512      # per-core head-group q dims (8 heads x 64)
DO = 512      # per-core output channels (half of 1024)
NJ = 4        # 512-wide t-blocks
NTB = 4       # projection t-blocks
NI = TKV // P  # 16 kv chunks
NHP = DQ // P  # 4 head-pairs
CQ_CH = CQ // P   # 8
CKV_CH = CKV // P  # 6
NCC = 2 * NHP     # 8 dc chunks in the gathered attention output

F32 = mybir.dt.float32
F32R = mybir.dt.float32r
EXP = mybir.ActivationFunctionType.Exp
ADD = mybir.AluOpType.add
MUL = mybir.AluOpType.mult

_compiled = None


def _build():
    nc = bacc.Bacc("TRN2", target_bir_lowering=False, debug=False,
                   num_devices=N_CORES)

    xqT = nc.dram_tensor("xqT", [CQ, TQ], F32R, kind="ExternalInput")
    xkvT = nc.dram_tensor("xkvT", [CKV, TKV], F32R, kind="ExternalInput")
    wqT = nc.dram_tensor("wqT", [CQ, DQ], F32R, kind="ExternalInput")
    wkT = nc.dram_tensor("wkT", [CKV, DQ], F32R, kind="ExternalInput")
    wvT = nc.dram_tensor("wvT", [CKV, DQ], F32R, kind="ExternalInput")
    # full-dc Wo slice for this core's output-channel half, dc rows in
    # gathered order (head-group 0 rows then head-group 1 rows)
    woT = nc.dram_tensor("woT", [2 * DQ, DO], F32R, kind="ExternalInput")
    onesc = nc.dram_tensor("onesc", [P, 8], F32R, kind="ExternalInput")
    out_ext = nc.dram_tensor("out", [DO, TQ], F32, kind="ExternalOutput")

    groups = [[2 * b, 2 * b + 1] for b in range(N_CORES // 2)]

    with tile.TileContext(nc) as tc:
        with (
            tc.tile_pool(name="weights", bufs=1) as wpool,
            tc.tile_pool(name="xload", bufs=1) as xpool,
            tc.tile_pool(name="stage", bufs=1) as stpool,
            tc.tile_pool(name="attn", bufs=1) as apool,
            tc.tile_pool(name="dram", bufs=1, space="DRAM") as dpool,
        ):
            # ---- resident weights ----
            wq_sb = wpool.tile([P, CQ_CH, DQ], F32R, tag="wq")
            wk_sb = wpool.tile([P, CKV_CH, DQ], F32R, tag="wk")
            wv_sb = wpool.tile([P, CKV_CH, DQ], F32R, tag="wv")
            wo_sb = wpool.tile([P, NCC, DO], F32R, tag="wo")
            nc.sync.dma_start(wq_sb[:], wqT.ap().rearrange("(n p) d -> p n d", p=P))
            nc.sync.dma_start(wk_sb[:], wkT.ap().rearrange("(n p) d -> p n d", p=P))
            nc.sync.dma_start(wv_sb[:], wvT.ap().rearrange("(n p) d -> p n d", p=P))
            nc.sync.dma_start(wo_sb[:], woT.ap().rearrange("(n p) d -> p n d", p=P))
            ones_sb = wpool.tile([P, 8, 1], F32R, tag="ones")
            nc.sync.dma_start(ones_sb[:],
                              onesc.ap().rearrange("p (n o) -> p n o", o=1))

            # ---- internal DRAM ----
            qT_d = dpool.tile([NHP, P, TQ], F32R, tag="qT_d")
            kT_d = dpool.tile([NHP, P, TKV], F32R, tag="kT_d")
            v_d = dpool.tile([TKV, 8 * 65], F32R, tag="v_d")
            ag_in = [dpool.tile([P, TQ], F32R, tag=f"agi{h}", name=f"agi{h}")
                     for h in range(NHP)]
            ag_out = [dpool.tile([2, P, TQ], F32R, tag=f"ago{h}",
                                 name=f"ago{h}")
                      for h in range(NHP)]

            xq_r = xqT.ap().rearrange("(n p) t -> p n t", p=P)
            xkv_r = xkvT.ap().rearrange("(n p) t -> p n t", p=P)
            v_r = v_d[:].rearrange("(n p) d -> p n d", p=P)

            # ================= projections =================
            with tc.tile_pool(name="psum_proj", bufs=4, space="PSUM") as ps_u:
                for tb in range(NTB):
                    ts_ = slice(tb * 512, (tb + 1) * 512)
                    xkv_t = []
                    for c in range(CKV_CH):
                        xkc = xpool.tile([P, 512], F32R, tag="xkv", bufs=8,
                                         name=f"xkv_{tb}_{c}")
                        nc.sync.dma_start(xkc[:], xkv_r[:, c, ts_])
                        xkv_t.append(xkc)
                    xq_t = []
                    for c in range(CQ_CH):
                        xqc = xpool.tile([P, 512], F32R, tag="xq", bufs=10,
                                         name=f"xq_{tb}_{c}")
                        nc.sync.dma_start(xqc[:], xq_r[:, c, ts_])
                        xq_t.append(xqc)

                    # V projection: v[t, dv] for the 4 t-chunks of this block
                    for s in range(4):
                        tc_i = tb * 4 + s
                        pv = ps_u.tile([P, 512], F32, tag="u")
                        for c in range(CKV_CH):
                            nc.tensor.matmul(
                                pv[:], xkv_t[c][:, s * P:(s + 1) * P],
                                wv_sb[:, c, :],
                                start=(c == 0), stop=(c == CKV_CH - 1))
                        vst = stpool.tile([P, 8, 65], F32R, tag="vstage",
                                          bufs=2)
                        nc.vector.tensor_copy(
                            vst[:, :, 0:64],
                            pv[:].rearrange("p (h d) -> p h d", d=64))
                        nc.vector.tensor_copy(vst[:, :, 64:65], ones_sb[:])
                        nc.sync.dma_start(v_d[tc_i * P:(tc_i + 1) * P, :],
                                          vst[:])

                    # K/Q projections into DRAM (channel-major, per head-pair)
                    for hp in range(NHP):
                        hs = slice(hp * P, (hp + 1) * P)
                        pk = ps_u.tile([P, 512], F32, tag="u")
                        for c in range(CKV_CH):
                            nc.tensor.matmul(
                                pk[:], wk_sb[:, c, hs], xkv_t[c][:],
                                start=(c == 0), stop=(c == CKV_CH - 1))
                        kst = stpool.tile([P, 512], F32R, tag="kqstage",
                                          bufs=3)
                        nc.vector.tensor_copy(kst[:], pk[:])
                        nc.sync.dma_start(kT_d[hp, :, ts_], kst[:])

                        pq = ps_u.tile([P, 512], F32, tag="u")
                        for c in range(CQ_CH):
                            nc.tensor.matmul(
                                pq[:], wq_sb[:, c, hs], xq_t[c][:],
                                start=(c == 0), stop=(c == CQ_CH - 1))
                        qst = stpool.tile([P, 512], F32R, tag="kqstage",
                                          bufs=3)
                        nc.vector.tensor_copy(qst[:], pq[:])
                        nc.sync.dma_start(qT_d[hp, :, ts_], qst[:])

            # ========== attention, with per-head-pair AllGather ==========
            with (
                tc.tile_pool(name="psum_s", bufs=2, space="PSUM") as ps_s,
                tc.tile_pool(name="psum_pv", bufs=4, space="PSUM") as ps_pv,
            ):
                for hp in range(NHP):
                    kt = apool.tile([P, TKV], F32R, tag="kt", bufs=2)
                    nc.sync.dma_start(kt[:], kT_d[hp])
                    vh = apool.tile([P, NI, 130], F32R, tag="vh", bufs=2)
                    nc.sync.dma_start(vh[:], v_r[:, :, hp * 130:(hp + 1) * 130])
                    ao = apool.tile([P, TQ], F32R, tag="ao", bufs=2)
                    for j in range(NJ):
                        js = slice(j * 512, (j + 1) * 512)
                        qt = apool.tile([P, 512], F32R, tag="qt", bufs=3)
                        nc.sync.dma_start(qt[:], qT_d[hp, :, js])
                        acc_a = ps_pv.tile([P, 512], F32, tag="pv")
                        acc_b = ps_pv.tile([P, 512], F32, tag="pv")
                        for i in range(NI):
                            isl = slice(i * P, (i + 1) * P)
                            sc = ps_s.tile([P, 1024], F32, tag="sc")
                            nc.tensor.matmul(sc[:, 0:512], kt[0:64, isl],
                                             qt[0:64, :], start=True,
                                             stop=True)
                            nc.tensor.matmul(sc[:, 512:1024], kt[64:128, isl],
                                             qt[64:128, :], start=True,
                                             stop=True)
                            ex = stpool.tile([P, 1024], F32R, tag="ex", bufs=3)
                            nc.scalar.activation(ex[:], sc[:], EXP, scale=0.125)
                            nc.tensor.matmul(acc_a[0:65, :], vh[:, i, 0:65],
                                             ex[:, 0:512],
                                             start=(i == 0), stop=(i == NI - 1))
                            nc.tensor.matmul(acc_b[0:65, :], vh[:, i, 65:130],
                                             ex[:, 512:1024],
                                             start=(i == 0), stop=(i == NI - 1))
                        # normalize: ao[:, js] = acc[0:64] / acc[64]
                        for half, acc in ((0, acc_a), (1, acc_b)):
                            rec = stpool.tile([P, 512], F32, tag="rec", bufs=2)
                            nc.vector.reciprocal(rec[0:1, :], acc[64:65, :])
                            bc = stpool.tile([P, 512], F32, tag="bc", bufs=2)
                            nc.gpsimd.partition_broadcast(bc[0:64, :],
                                                          rec[0:1, :],
                                                          channels=64)
                            nc.vector.tensor_tensor(
                                ao[half * 64:(half + 1) * 64, js],
                                acc[0:64, :], bc[0:64, :], op=MUL)
                    # exchange this head-pair's attention output with the
                    # pair peer while later head-pairs keep computing
                    nc.sync.dma_start(ag_in[hp][:], ao[:])
                    nc.gpsimd.collective_compute(
                        "AllGather", mybir.AluOpType.bypass,
                        replica_groups=groups,
                        ins=[ag_in[hp].opt()], outs=[ag_out[hp].opt()])

                # ===== output projection (my half of the Wo columns) =====
                # cc chunk order puts the hp=3 chunks last so most of each
                # accumulation can run before the final AllGather lands
                cc_order = [(g, hp) for hp in range(NHP) for g in range(2)]
                for j in range(NJ):
                    js = slice(j * 512, (j + 1) * 512)
                    rhs = []
                    for g, hp in cc_order:
                        aog = stpool.tile([P, 512], F32R, tag="aog", bufs=6,
                                          name=f"aog_{j}_{g}_{hp}")
                        nc.sync.dma_start(aog[:], ag_out[hp][g, :, js])
                        rhs.append(aog)
                    for do in range(DO // P):
                        po = ps_pv.tile([P, 512], F32, tag="pv")
                        for n, (g, hp) in enumerate(cc_order):
                            cc = g * NHP + hp
                            nc.tensor.matmul(
                                po[:], wo_sb[:, cc, do * P:(do + 1) * P],
                                rhs[n][:],
                                start=(n == 0), stop=(n == NCC - 1))
                        ost = stpool.tile([P, 512], F32, tag="ost", bufs=3)
                        nc.vector.tensor_copy(ost[:], po[:])
                        nc.sync.dma_start(out_ext[do * P:(do + 1) * P, js],
                                          ost[:])

    nc.compile()
    return nc


def make_in_maps(q_tokens, kv_tokens, Wq, Wk, Wv, Wo):
    q_tokens = np.asarray(q_tokens, np.float32)
    kv_tokens = np.asarray(kv_tokens, np.float32)
    Wq = np.asarray(Wq, np.float32)
    Wk = np.asarray(Wk, np.float32)
    Wv = np.asarray(Wv, np.float32)
    Wo = np.asarray(Wo, np.float32)
    in_maps = []
    for c in range(N_CORES):
        b, hg = c // 2, c % 2
        sl = slice(hg * DQ, (hg + 1) * DQ)
        osl = slice(hg * DO, (hg + 1) * DO)
        in_maps.append({
            "xqT": np.ascontiguousarray(q_tokens[b].T),
            "xkvT": np.ascontiguousarray(kv_tokens[b].T),
            "wqT": np.ascontiguousarray(Wq[sl, :].T),
            "wkT": np.ascontiguousarray(Wk[sl, :].T),
            "wvT": np.ascontiguousarray(Wv[sl, :].T),
            # [dc, do-half] with dc rows in gathered (global head) order
            "woT": np.ascontiguousarray(Wo[osl, :].T),
            "onesc": np.ones((P, 8), np.float32),
        })
    return in_maps


def kernel(q_tokens, kv_tokens, Wq, Wk, Wv, Wo):
    global _compiled
    if _compiled is None:
        _compiled = _build()
    nc = _compiled

    in_maps = make_in_maps(q_tokens, kv_tokens, Wq, Wk, Wv, Wo)
    res = bass_utils.run_bass_kernel_spmd(nc, in_maps,
                                          core_ids=list(range(N_CORES)))
    B = 4
    out = np.empty((B, TQ, 2 * DO), np.float32)
    for c in range(N_CORES):
        b, hg = c // 2, c % 2
        out[b, :, hg * DO:(hg + 1) * DO] = res.results[c]["out"].T
    return out
